# revision 19
# baseline (speedup 1.0000x reference)
"""Depthwise-separable conv2d block (dw3x3 + BN + ReLU + map-cut, pw1x1 + BN +
ReLU + map-cut) on 8 Trainium2 NeuronCores, data-parallel over the batch dim.

Fixed problem shapes: x (32,256,56,56) f32 -> out (32,512,54,54) f32.

Per-core device program (4 images each), fp8e4m3 matmul operands with
DoubleRow perf mode (two 128-deep contraction subtiles per instruction),
f32 PSUM:
  - depthwise 3x3 VALID conv: per 9-output-row chunk, 5 PSUM-accumulated
    DoubleRow matmuls, each packing TWO taps of the 3x3 stencil as the two
    k-subtiles (pairs chosen so the SBUF pair stride is even: odd strides
    fault the PE's DR ifmap fetch).  The rhs streams 504 contiguous
    elements (9 rows x 56 incl. 2 wrap cols); wrap cols land in psum cols
    the drains never read.  The 5th pair carries (1,1) plus a BIAS tap
    (all-ones rhs region times diag(b1)), so drains are a pure relu.
  - dw drains: relu(psum) -> fp8 Y (the pw DR rhs layout), split DVE
    (cin tile 0) / ACT (cin tile 1); per-map max via one DVE reduce per
    (image, cin tile) feeds the dw cut.  The cut is folded into
    per-image masked pw weights w2m = w2 * mask (and |w2| * mask).
  - pw cut masks are computed BEFORE the pw GEMM from the safe upper
    bound zUb[o] = sum_c |w2m[c,o]| * max_y[c] >= max_n |z'[o,n]| via one
    tiny DoubleRow matvec per (image, cout tile) into spare psum cols.
    mask2 = (zUb + b2 >= thresh) can only false-KEEP maps whose true
    values are < thresh (error <= 1e-3 abs), never false-cut, and keeps
    the all-zero canonical output exact.
  - pointwise 1x1: one DoubleRow matmul per 486-col chunk (K=256 = both
    cin tiles as the two k-subtiles); ACT drains each 2-chunk psum group
    once with relu + scale=mask2 + bias=mask2*b2 fused -> final bf16 z,
    stored per group (host upcasts to f32).  No second pass over z.
Pitfalls honored: no gpsimd bulk math (it is a slow DSP), no per-partition
AP scalars on bf16/fp8 DVE ops (PTR operand fetch is ~100x slow; masked
multiplies use stride-0 broadcast tensor operands instead), matmul out
<= one psum bank, DR pair strides even.
BatchNorm (inference) is folded into the conv weights/biases on the host.
"""

import os

os.environ.setdefault("NEURON_RT_RESET_CORES", "1")

import ml_dtypes
import numpy as np

import concourse.bacc as bacc
import concourse.bass as bass
import concourse.mybir as mybir
import concourse.tile as tile
from concourse.ap import AP
from concourse.bass_utils import run_bass_kernel_spmd

EPS = 1e-5
DW_THRESH = 4.0
PW_THRESH = 0.001

B, CIN, COUT, H, W = 32, 256, 512, 56, 56
HO, WO = 54, 54
NPIX = HO * WO          # 2916
NCORES = 8
BPC = B // NCORES       # 4 images per core
P = 128                 # partitions
KT = CIN // P           # 2 cin tiles
MT = COUT // P          # 4 cout tiles
NCH = 6                 # chunks per map: 6 x (9 rows x 54 cols)
CHROWS = HO // NCH      # 9
CHUNK = CHROWS * WO     # 486 output cols per chunk
FLAT = CHROWS * W       # 504 streamed cols per chunk (incl. 2 wrap cols)
XLEN = H * W            # 3136
ONES0 = XLEN + 1        # odd base of the ones region (bias tap rhs)
XPAD = XLEN + 8 + FLAT  # fp8 image + ones pad
BANK = 512

F32 = mybir.dt.float32
FP8 = mybir.dt.float8e4
BF16 = mybir.dt.bfloat16
DR = mybir.MatmulPerfMode.DoubleRow
ALU = mybir.AluOpType
AF = mybir.ActivationFunctionType

# tap pairs for the 5 DoubleRow depthwise matmuls (flat offset of tap
# (di,dj) = 56*di + dj; the pair stride o1-o0 must be EVEN).
TAP_PAIRS = [((0, 0), (0, 2)), ((1, 0), (1, 2)), ((2, 0), (2, 2)),
             ((0, 1), (2, 1)), ((1, 1), None)]

_cached_nc = None


def _build_program():
    nc = bacc.Bacc("TRN2", target_bir_lowering=False, debug=False)

    xs = nc.dram_tensor("xs", [BPC, CIN, XLEN], FP8, kind="ExternalInput").ap()
    dwp = nc.dram_tensor("dwp", [P, KT, 5, 2, P], FP8, kind="ExternalInput").ap()
    w2t = nc.dram_tensor("w2t", [P, KT, COUT], FP8, kind="ExternalInput").ap()
    uw2t = nc.dram_tensor("uw2t", [P, KT, COUT], FP8, kind="ExternalInput").ap()
    b2t = nc.dram_tensor("b2t", [P, MT], F32, kind="ExternalInput").ap()
    zs = nc.dram_tensor("zs", [BPC, COUT, NPIX], BF16, kind="ExternalOutput").ap()

    with tile.TileContext(nc) as tc:
        with (
            tc.tile_pool(name="consts", bufs=1) as consts,
            tc.tile_pool(name="xp", bufs=8) as xp,
            tc.tile_pool(name="yp", bufs=2) as yp,
            tc.tile_pool(name="zp", bufs=3) as zp,
            tc.tile_pool(name="wm", bufs=2) as wmp,
            tc.tile_pool(name="st", bufs=48) as st,
            tc.tile_pool(name="ps", bufs=4, space="PSUM") as psp,
        ):
            xtiles = {}

            def load_x(b, k):
                X = xp.tile([P, XPAD], FP8, name="X")
                nc.sync.dma_start(out=X[:, 0:XLEN],
                                  in_=xs[b, k * P:(k + 1) * P, :])
                # ones region for the bias tap (covers any chunk's base)
                nc.gpsimd.memset(X[:, XLEN:XPAD], 1.0)
                xtiles[b, k] = X

            # first image's inputs + dw weights gate the first matmul
            load_x(0, 0)
            dwsb = consts.tile([P, KT, 5, 2, P], FP8)
            nc.sync.dma_start(out=dwsb, in_=dwp)
            load_x(0, 1)
            w2sb = consts.tile([P, KT, COUT], FP8)
            nc.sync.dma_start(out=w2sb, in_=w2t)
            uw2sb = consts.tile([P, KT, COUT], FP8)
            nc.sync.dma_start(out=uw2sb, in_=uw2t)
            b2sb = consts.tile([P, MT], F32)
            nc.sync.dma_start(out=b2sb, in_=b2t)
            for b in range(BPC):
                for k in range(KT):
                    if (b, k) not in xtiles:
                        load_x(b, k)

            def dw_rhs(X, pair, n):
                """[P, 2, 504] DoubleRow rhs: two tap-shifted flat views."""
                t0, t1 = TAP_PAIRS[pair]
                o0 = FLAT * n + W * t0[0] + t0[1]
                o1 = ONES0 if t1 is None else FLAT * n + W * t1[0] + t1[1]
                return AP(X.tensor, X.offset + o0,
                          [list(X.ap[0]), [o1 - o0, 2], [1, FLAT]])

            def emit_dw_group(b, k, g, Yk3):
                X = xtiles[b, k]
                P1 = psp.tile([P, 2, BANK], F32, name="P2")
                for j in range(2):
                    n = 2 * g + j
                    for p in range(5):
                        nc.tensor.matmul(
                            P1[:, j, 0:FLAT], lhsT=dwsb[:, k, p],
                            rhs=dw_rhs(X, p, n),
                            start=(p == 0), stop=(p == 4), perf_mode=DR)
                    # drain valid cols (skip the 2 wrap cols per row):
                    # y = relu(psum) -> fp8 (b1 was added by the bias tap)
                    src = P1[:, j, 0:FLAT].rearrange(
                        "p (r w) -> p r w", w=W)[:, :, 0:WO]
                    dst = Yk3[:, n].rearrange("p (r w) -> p r w", w=WO)
                    if k == 0:
                        nc.vector.tensor_scalar(
                            out=dst, in0=src, scalar1=0.0, scalar2=None,
                            op0=ALU.max)
                    else:
                        nc.scalar.activation(out=dst, in_=src, func=AF.Relu,
                                             bias=0.0, scale=1.0)

            Ys = {}
            masks = {}

            def dw_phase(b):
                Yi = yp.tile([P, KT, NPIX], FP8, name="Yi")
                Yi3 = Yi.rearrange("p k (c x) -> p k c x", x=CHUNK)
                w2m = wmp.tile([P, KT, COUT], FP8, name="w2m")
                uw2m = wmp.tile([P, KT, COUT], FP8, name="uw2m")
                Mv = st.tile([P, 4], FP8, name="Mv")
                for k in range(KT):
                    for g in range(NCH // 2):
                        emit_dw_group(b, k, g, Yi3[:, k])
                    # dw cut: zero the (image, channel) map if max(y) < 4
                    M = st.tile([P, 1], F32, name="M")
                    nc.vector.tensor_reduce(
                        M, Yi[:, k], axis=mybir.AxisListType.X, op=ALU.max)
                    mask1 = st.tile([P, 1], F32, name="mask1")
                    nc.vector.tensor_scalar(
                        out=mask1, in0=M, scalar1=DW_THRESH, scalar2=None,
                        op0=ALU.is_ge)
                    # masked pw weights (stride-0 broadcast operand: a
                    # per-partition AP scalar would hit the slow PTR path)
                    m1b = mask1.to_broadcast([P, COUT])
                    nc.vector.tensor_tensor(
                        out=w2m[:, k], in0=w2sb[:, k], in1=m1b, op=ALU.mult)
                    nc.vector.tensor_tensor(
                        out=uw2m[:, k], in0=uw2sb[:, k], in1=m1b, op=ALU.mult)
                    # per-channel y max, inflated 1.125x to stay an upper
                    # bound after fp8 rounding, at even slot stride for DR
                    nc.vector.tensor_scalar(
                        out=Mv[:, 2 * k:2 * k + 1], in0=M, scalar1=1.125,
                        scalar2=None, op0=ALU.mult)
                Ys[b] = (Yi, Yi3, w2m, uw2m, Mv)

            def pw_phase(b):
                Yi, Yi3, w2m, uw2m, Mv = Ys[b]
                # pw upper-bound matvecs: zUb[o] = sum_c |w2m[c,o]|*Mv[c],
                # all four into spare psum cols of ONE tile (m=0's first
                # group) so mask2/bias are ready before any pw drain
                G0 = psp.tile([P, 2, BANK], F32, name="P2")
                rhs = AP(Mv.tensor, Mv.offset,
                         [list(Mv.ap[0]), [2, 2], [1, 1]])
                for m in range(MT):
                    c = CHUNK + 8 + m
                    nc.tensor.matmul(
                        G0[:, 0, c:c + 1],
                        lhsT=uw2m[:, :, m * P:(m + 1) * P], rhs=rhs,
                        start=True, stop=True, perf_mode=DR)
                mask2 = st.tile([P, MT], F32, name="mask2")
                b2m = st.tile([P, MT], F32, name="b2m")
                PU = st.tile([P, MT], F32, name="PU")
                nc.vector.tensor_scalar(
                    out=PU, in0=G0[:, 0, CHUNK + 8:CHUNK + 8 + MT],
                    scalar1=0.0, scalar2=None, op0=ALU.add)
                nc.vector.tensor_tensor(
                    out=mask2, in0=PU, in1=b2sb, op=ALU.add)
                nc.vector.tensor_scalar(
                    out=mask2, in0=mask2, scalar1=PW_THRESH, scalar2=None,
                    op0=ALU.is_ge)
                nc.vector.tensor_tensor(
                    out=b2m, in0=b2sb, in1=mask2, op=ALU.mult)

                for m in range(MT):
                    zf = zp.tile([P, NPIX], BF16, name="zf")
                    lhsT = w2m[:, :, m * P:(m + 1) * P]
                    for g in range(3):
                        P2 = G0 if (m == 0 and g == 0) else psp.tile(
                            [P, 2, BANK], F32, name="P2")
                        for j in range(2):
                            n = 2 * g + j
                            nc.tensor.matmul(
                                P2[:, j, 0:CHUNK], lhsT=lhsT,
                                rhs=Yi3[:, :, n],
                                start=True, stop=True, perf_mode=DR)
                        # single fused drain: relu(mask2*psum + mask2*b2)
                        # = mask2 * relu(psum + b2) -> final bf16 z
                        dst = zf[:, g * 2 * CHUNK:(g + 1) * 2 * CHUNK] \
                            .rearrange("p (c x) -> p c x", x=CHUNK)
                        nc.scalar.activation(
                            out=dst, in_=P2[:, :, 0:CHUNK], func=AF.Relu,
                            bias=b2m[:, m:m + 1], scale=mask2[:, m:m + 1])
                        nc.sync.dma_start(
                            out=zs[b, m * P:(m + 1) * P,
                                   g * 2 * CHUNK:(g + 1) * 2 * CHUNK],
                            in_=zf[:, g * 2 * CHUNK:(g + 1) * 2 * CHUNK])

            # skewed emission: dw(b+1) is queued before pw(b) so the
            # in-order tensor queue never stalls on image b's cut masks
            dw_phase(0)
            for b in range(BPC):
                if b + 1 < BPC:
                    dw_phase(b + 1)
                pw_phase(b)
    nc.compile()
    return nc


def _prep_params(dw_w, dw_b, dw_gamma, dw_beta, dw_mean, dw_var,
                 pw_w, pw_b, pw_gamma, pw_beta, pw_mean, pw_var):
    dw_scale = dw_gamma / np.sqrt(dw_var + EPS)
    b1 = dw_b * dw_scale + dw_beta - dw_mean * dw_scale          # (256,)
    w1 = dw_w[:, 0] * dw_scale[:, None, None]                    # (256,3,3)

    dwp = np.zeros((P, KT, 5, 2, P), np.float32)
    idx = np.arange(P)
    for k in range(KT):
        for p, (t0, t1) in enumerate(TAP_PAIRS):
            dwp[idx, k, p, 0, idx] = w1[k * P:(k + 1) * P, t0[0], t0[1]]
            if t1 is None:
                dwp[idx, k, p, 1, idx] = b1[k * P:(k + 1) * P]
            else:
                dwp[idx, k, p, 1, idx] = w1[k * P:(k + 1) * P, t1[0], t1[1]]

    pw_scale = pw_gamma / np.sqrt(pw_var + EPS)
    b2 = pw_b * pw_scale + pw_beta - pw_mean * pw_scale          # (512,)
    w2 = pw_w * pw_scale[:, None]                                # (512,256)
    # w2t[ck, k, o] = w2[o, k*128+ck]
    w2t = np.ascontiguousarray(
        w2.T.reshape(KT, P, COUT).transpose(1, 0, 2)).astype(np.float32)
    b2t = np.ascontiguousarray(b2.reshape(MT, P).T).astype(np.float32)
    w2t8 = w2t.astype(ml_dtypes.float8_e4m3)
    uw2t8 = np.abs(w2t8.astype(np.float32)).astype(ml_dtypes.float8_e4m3)
    return (dwp.astype(ml_dtypes.float8_e4m3), w2t8, uw2t8, b2t)


def kernel(x, dw_w, dw_b, dw_gamma, dw_beta, dw_mean, dw_var,
           pw_w, pw_b, pw_gamma, pw_beta, pw_mean, pw_var):
    global _cached_nc
    x = np.ascontiguousarray(np.asarray(x, np.float32))
    args = [np.asarray(a, np.float32) for a in
            (dw_w, dw_b, dw_gamma, dw_beta, dw_mean, dw_var,
             pw_w, pw_b, pw_gamma, pw_beta, pw_mean, pw_var)]
    dwp, w2t, uw2t, b2t = _prep_params(*args)
    x8 = x.reshape(B, CIN, XLEN).astype(ml_dtypes.float8_e4m3)

    if _cached_nc is None:
        _cached_nc = _build_program()
    nc = _cached_nc

    in_maps = []
    for c in range(NCORES):
        in_maps.append({
            "xs": np.ascontiguousarray(x8[c * BPC:(c + 1) * BPC]),
            "dwp": dwp,
            "w2t": w2t,
            "uw2t": uw2t,
            "b2t": b2t,
        })
    res = run_bass_kernel_spmd(nc, in_maps, core_ids=list(range(NCORES)))
    out = np.concatenate(
        [res.results[c]["zs"].astype(np.float32) for c in range(NCORES)],
        axis=0)
    return out.reshape(B, COUT, HO, WO)


# revision 20
# speedup vs baseline: 1.0263x; 1.0263x over previous
"""Depthwise-separable conv2d block (dw3x3 + BN + ReLU + map-cut, pw1x1 + BN +
ReLU + map-cut) on 8 Trainium2 NeuronCores, data-parallel over the batch dim.

Fixed problem shapes: x (32,256,56,56) f32 -> out (32,512,54,54) f32.

Per-core device program (4 images each), fp8e4m3 matmul operands with
DoubleRow perf mode (two 128-deep contraction subtiles per instruction),
f32 PSUM:
  - depthwise 3x3 VALID conv: per 9-output-row chunk, 5 PSUM-accumulated
    DoubleRow matmuls, each packing TWO taps of the 3x3 stencil as the two
    k-subtiles (pairs chosen so the SBUF pair stride is even: odd strides
    fault the PE's DR ifmap fetch).  The rhs streams 504 contiguous
    elements (9 rows x 56 incl. 2 wrap cols); wrap cols land in psum cols
    the drains never read.  The 5th pair carries (1,1) plus a BIAS tap
    (all-ones rhs region times diag(b1)), so drains are a pure relu.
  - dw drains: relu(psum) -> fp8 Y (the pw DR rhs layout), split DVE
    (cin tile 0) / ACT (cin tile 1); per-map max via one DVE reduce per
    (image, cin tile) feeds the dw cut.  The cut is folded into
    per-image masked pw weights w2m = w2 * mask (and |w2| * mask).
  - pw cut masks are computed BEFORE the pw GEMM from the safe upper
    bound zUb[o] = sum_c |w2m[c,o]| * max_y[c] >= max_n |z'[o,n]| via one
    tiny DoubleRow matvec per (image, cout tile) into spare psum cols.
    mask2 = (zUb + b2 >= thresh) can only false-KEEP maps whose true
    values are < thresh (error <= 1e-3 abs), never false-cut, and keeps
    the all-zero canonical output exact.
  - pointwise 1x1: one DoubleRow matmul per 486-col chunk (K=256 = both
    cin tiles as the two k-subtiles); ACT drains each 2-chunk psum group
    once with relu + scale=mask2 + bias=mask2*b2 fused -> final bf16 z,
    stored per group (host upcasts to f32).  No second pass over z.
Pitfalls honored: no gpsimd bulk math (it is a slow DSP), no per-partition
AP scalars on bf16/fp8 DVE ops (PTR operand fetch is ~100x slow; masked
multiplies use stride-0 broadcast tensor operands instead), matmul out
<= one psum bank, DR pair strides even.
BatchNorm (inference) is folded into the conv weights/biases on the host.
"""

import os

os.environ.setdefault("NEURON_RT_RESET_CORES", "1")

import ml_dtypes
import numpy as np

import concourse.bacc as bacc
import concourse.bass as bass
import concourse.mybir as mybir
import concourse.tile as tile
from concourse.ap import AP
from concourse.bass_utils import run_bass_kernel_spmd

EPS = 1e-5
DW_THRESH = 4.0
PW_THRESH = 0.001

B, CIN, COUT, H, W = 32, 256, 512, 56, 56
HO, WO = 54, 54
NPIX = HO * WO          # 2916
NCORES = 8
BPC = B // NCORES       # 4 images per core
P = 128                 # partitions
KT = CIN // P           # 2 cin tiles
MT = COUT // P          # 4 cout tiles
NCH = 6                 # chunks per map: 6 x (9 rows x 54 cols)
CHROWS = HO // NCH      # 9
CHUNK = CHROWS * WO     # 486 output cols per chunk
FLAT = CHROWS * W       # 504 streamed cols per chunk (incl. 2 wrap cols)
XLEN = H * W            # 3136
ONES0 = XLEN + 1        # odd base of the ones region (bias tap rhs)
XPAD = XLEN + 8 + FLAT  # fp8 image + ones pad
BANK = 512

F32 = mybir.dt.float32
FP8 = mybir.dt.float8e4
BF16 = mybir.dt.bfloat16
DR = mybir.MatmulPerfMode.DoubleRow
ALU = mybir.AluOpType
AF = mybir.ActivationFunctionType

# tap pairs for the 5 DoubleRow depthwise matmuls (flat offset of tap
# (di,dj) = 56*di + dj; the pair stride o1-o0 must be EVEN).
TAP_PAIRS = [((0, 0), (0, 2)), ((1, 0), (1, 2)), ((2, 0), (2, 2)),
             ((0, 1), (2, 1)), ((1, 1), None)]

_cached_nc = None


def _build_program():
    nc = bacc.Bacc("TRN2", target_bir_lowering=False, debug=False)

    xs = nc.dram_tensor("xs", [BPC, CIN, XLEN], FP8, kind="ExternalInput").ap()
    dwp = nc.dram_tensor("dwp", [P, KT, 5, 2, P], FP8, kind="ExternalInput").ap()
    w2t = nc.dram_tensor("w2t", [P, KT, COUT], FP8, kind="ExternalInput").ap()
    uw2t = nc.dram_tensor("uw2t", [P, KT, COUT], FP8, kind="ExternalInput").ap()
    b2t = nc.dram_tensor("b2t", [P, MT], F32, kind="ExternalInput").ap()
    zs = nc.dram_tensor("zs", [BPC, COUT, NPIX], BF16, kind="ExternalOutput").ap()

    with tile.TileContext(nc) as tc:
        with (
            tc.tile_pool(name="consts", bufs=1) as consts,
            tc.tile_pool(name="xp", bufs=8) as xp,
            tc.tile_pool(name="yp", bufs=2) as yp,
            tc.tile_pool(name="zp", bufs=3) as zp,
            tc.tile_pool(name="wm", bufs=2) as wmp,
            tc.tile_pool(name="st", bufs=48) as st,
            tc.tile_pool(name="psdw", bufs=4, space="PSUM") as psdw,
            tc.tile_pool(name="pspw", bufs=2, space="PSUM") as pspw,
        ):
            xtiles = {}

            def load_x(b, k):
                X = xp.tile([P, XPAD], FP8, name="X")
                nc.sync.dma_start(out=X[:, 0:XLEN],
                                  in_=xs[b, k * P:(k + 1) * P, :])
                # ones region for the bias tap (covers any chunk's base)
                nc.gpsimd.memset(X[:, XLEN:XPAD], 1.0)
                xtiles[b, k] = X

            # first image's inputs + dw weights gate the first matmul
            load_x(0, 0)
            dwsb = consts.tile([P, KT, 5, 2, P], FP8)
            nc.sync.dma_start(out=dwsb, in_=dwp)
            load_x(0, 1)
            w2sb = consts.tile([P, KT, COUT], FP8)
            nc.sync.dma_start(out=w2sb, in_=w2t)
            uw2sb = consts.tile([P, KT, COUT], FP8)
            nc.sync.dma_start(out=uw2sb, in_=uw2t)
            b2sb = consts.tile([P, MT], F32)
            nc.sync.dma_start(out=b2sb, in_=b2t)
            for b in range(BPC):
                for k in range(KT):
                    if (b, k) not in xtiles:
                        load_x(b, k)

            def dw_rhs(X, pair, n):
                """[P, 2, 504] DoubleRow rhs: two tap-shifted flat views."""
                t0, t1 = TAP_PAIRS[pair]
                o0 = FLAT * n + W * t0[0] + t0[1]
                o1 = ONES0 if t1 is None else FLAT * n + W * t1[0] + t1[1]
                return AP(X.tensor, X.offset + o0,
                          [list(X.ap[0]), [o1 - o0, 2], [1, FLAT]])

            def emit_dw_chunk(b, k, n, Yk3):
                X = xtiles[b, k]
                P1 = psdw.tile([P, BANK], F32, name="P1")
                for p in range(5):
                    nc.tensor.matmul(
                        P1[:, 0:FLAT], lhsT=dwsb[:, k, p],
                        rhs=dw_rhs(X, p, n),
                        start=(p == 0), stop=(p == 4), perf_mode=DR)
                # drain valid cols (skip the 2 wrap cols per row):
                # y = relu(psum) -> fp8  (b1 was added by the bias tap)
                src = P1[:, 0:FLAT].rearrange(
                    "p (r w) -> p r w", w=W)[:, :, 0:WO]
                dst = Yk3[:, n].rearrange("p (r w) -> p r w", w=WO)
                if k == 0:
                    nc.vector.tensor_scalar(
                        out=dst, in0=src, scalar1=0.0, scalar2=None,
                        op0=ALU.max)
                else:
                    nc.scalar.activation(out=dst, in_=src, func=AF.Relu,
                                         bias=0.0, scale=1.0)

            Ys = {}
            masks = {}

            def dw_phase(b):
                Yi = yp.tile([P, KT, NPIX], FP8, name="Yi")
                Yi3 = Yi.rearrange("p k (c x) -> p k c x", x=CHUNK)
                w2m = wmp.tile([P, KT, COUT], FP8, name="w2m")
                uw2m = wmp.tile([P, KT, COUT], FP8, name="uw2m")
                Mv = st.tile([P, 4], FP8, name="Mv")
                for k in range(KT):
                    for n in range(NCH):
                        emit_dw_chunk(b, k, n, Yi3[:, k])
                    # dw cut: zero the (image, channel) map if max(y) < 4
                    M = st.tile([P, 1], F32, name="M")
                    nc.vector.tensor_reduce(
                        M, Yi[:, k], axis=mybir.AxisListType.X, op=ALU.max)
                    mask1 = st.tile([P, 1], F32, name="mask1")
                    nc.vector.tensor_scalar(
                        out=mask1, in0=M, scalar1=DW_THRESH, scalar2=None,
                        op0=ALU.is_ge)
                    # masked pw weights (stride-0 broadcast operand: a
                    # per-partition AP scalar would hit the slow PTR path)
                    m1b = mask1.to_broadcast([P, COUT])
                    nc.vector.tensor_tensor(
                        out=w2m[:, k], in0=w2sb[:, k], in1=m1b, op=ALU.mult)
                    nc.vector.tensor_tensor(
                        out=uw2m[:, k], in0=uw2sb[:, k], in1=m1b, op=ALU.mult)
                    # per-channel y max, inflated 1.125x to stay an upper
                    # bound after fp8 rounding, at even slot stride for DR
                    nc.vector.tensor_scalar(
                        out=Mv[:, 2 * k:2 * k + 1], in0=M, scalar1=1.125,
                        scalar2=None, op0=ALU.mult)
                Ys[b] = (Yi, Yi3, w2m, uw2m, Mv)

            def pw_phase(b):
                Yi, Yi3, w2m, uw2m, Mv = Ys[b]
                # pw upper-bound matvecs: zUb[o] = sum_c |w2m[c,o]|*Mv[c],
                # all four into spare psum cols of ONE tile (m=0's first
                # group) so mask2/bias are ready before any pw drain
                G0 = pspw.tile([P, 2, BANK], F32, name="P2")
                rhs = AP(Mv.tensor, Mv.offset,
                         [list(Mv.ap[0]), [2, 2], [1, 1]])
                for m in range(MT):
                    c = CHUNK + 8 + m
                    nc.tensor.matmul(
                        G0[:, 0, c:c + 1],
                        lhsT=uw2m[:, :, m * P:(m + 1) * P], rhs=rhs,
                        start=True, stop=True, perf_mode=DR)
                mask2 = st.tile([P, MT], F32, name="mask2")
                b2m = st.tile([P, MT], F32, name="b2m")
                PU = st.tile([P, MT], F32, name="PU")
                nc.vector.tensor_scalar(
                    out=PU, in0=G0[:, 0, CHUNK + 8:CHUNK + 8 + MT],
                    scalar1=0.0, scalar2=None, op0=ALU.add)
                nc.vector.tensor_tensor(
                    out=mask2, in0=PU, in1=b2sb, op=ALU.add)
                nc.vector.tensor_scalar(
                    out=mask2, in0=mask2, scalar1=PW_THRESH, scalar2=None,
                    op0=ALU.is_ge)
                nc.vector.tensor_tensor(
                    out=b2m, in0=b2sb, in1=mask2, op=ALU.mult)

                for m in range(MT):
                    zf = zp.tile([P, NPIX], BF16, name="zf")
                    lhsT = w2m[:, :, m * P:(m + 1) * P]
                    for g in range(3):
                        P2 = G0 if (m == 0 and g == 0) else pspw.tile(
                            [P, 2, BANK], F32, name="P2")
                        for j in range(2):
                            n = 2 * g + j
                            nc.tensor.matmul(
                                P2[:, j, 0:CHUNK], lhsT=lhsT,
                                rhs=Yi3[:, :, n],
                                start=True, stop=True, perf_mode=DR)
                        # single fused drain: relu(mask2*psum + mask2*b2)
                        # = mask2 * relu(psum + b2) -> final bf16 z
                        dst = zf[:, g * 2 * CHUNK:(g + 1) * 2 * CHUNK] \
                            .rearrange("p (c x) -> p c x", x=CHUNK)
                        nc.scalar.activation(
                            out=dst, in_=P2[:, :, 0:CHUNK], func=AF.Relu,
                            bias=b2m[:, m:m + 1], scale=mask2[:, m:m + 1])
                        nc.sync.dma_start(
                            out=zs[b, m * P:(m + 1) * P,
                                   g * 2 * CHUNK:(g + 1) * 2 * CHUNK],
                            in_=zf[:, g * 2 * CHUNK:(g + 1) * 2 * CHUNK])

            # skewed emission: dw(b+1) is queued before pw(b) so the
            # in-order tensor queue never stalls on image b's cut masks
            dw_phase(0)
            for b in range(BPC):
                if b + 1 < BPC:
                    dw_phase(b + 1)
                pw_phase(b)
    nc.compile()
    return nc


def _prep_params(dw_w, dw_b, dw_gamma, dw_beta, dw_mean, dw_var,
                 pw_w, pw_b, pw_gamma, pw_beta, pw_mean, pw_var):
    dw_scale = dw_gamma / np.sqrt(dw_var + EPS)
    b1 = dw_b * dw_scale + dw_beta - dw_mean * dw_scale          # (256,)
    w1 = dw_w[:, 0] * dw_scale[:, None, None]                    # (256,3,3)

    dwp = np.zeros((P, KT, 5, 2, P), np.float32)
    idx = np.arange(P)
    for k in range(KT):
        for p, (t0, t1) in enumerate(TAP_PAIRS):
            dwp[idx, k, p, 0, idx] = w1[k * P:(k + 1) * P, t0[0], t0[1]]
            if t1 is None:
                dwp[idx, k, p, 1, idx] = b1[k * P:(k + 1) * P]
            else:
                dwp[idx, k, p, 1, idx] = w1[k * P:(k + 1) * P, t1[0], t1[1]]

    pw_scale = pw_gamma / np.sqrt(pw_var + EPS)
    b2 = pw_b * pw_scale + pw_beta - pw_mean * pw_scale          # (512,)
    w2 = pw_w * pw_scale[:, None]                                # (512,256)
    # w2t[ck, k, o] = w2[o, k*128+ck]
    w2t = np.ascontiguousarray(
        w2.T.reshape(KT, P, COUT).transpose(1, 0, 2)).astype(np.float32)
    b2t = np.ascontiguousarray(b2.reshape(MT, P).T).astype(np.float32)
    w2t8 = w2t.astype(ml_dtypes.float8_e4m3)
    uw2t8 = np.abs(w2t8.astype(np.float32)).astype(ml_dtypes.float8_e4m3)
    return (dwp.astype(ml_dtypes.float8_e4m3), w2t8, uw2t8, b2t)


def kernel(x, dw_w, dw_b, dw_gamma, dw_beta, dw_mean, dw_var,
           pw_w, pw_b, pw_gamma, pw_beta, pw_mean, pw_var):
    global _cached_nc
    x = np.ascontiguousarray(np.asarray(x, np.float32))
    args = [np.asarray(a, np.float32) for a in
            (dw_w, dw_b, dw_gamma, dw_beta, dw_mean, dw_var,
             pw_w, pw_b, pw_gamma, pw_beta, pw_mean, pw_var)]
    dwp, w2t, uw2t, b2t = _prep_params(*args)
    x8 = x.reshape(B, CIN, XLEN).astype(ml_dtypes.float8_e4m3)

    if _cached_nc is None:
        _cached_nc = _build_program()
    nc = _cached_nc

    in_maps = []
    for c in range(NCORES):
        in_maps.append({
            "xs": np.ascontiguousarray(x8[c * BPC:(c + 1) * BPC]),
            "dwp": dwp,
            "w2t": w2t,
            "uw2t": uw2t,
            "b2t": b2t,
        })
    res = run_bass_kernel_spmd(nc, in_maps, core_ids=list(range(NCORES)))
    out = np.concatenate(
        [res.results[c]["zs"].astype(np.float32) for c in range(NCORES)],
        axis=0)
    return out.reshape(B, COUT, HO, WO)


# revision 21
# speedup vs baseline: 1.1325x; 1.1035x over previous
"""Depthwise-separable conv2d block (dw3x3 + BN + ReLU + map-cut, pw1x1 + BN +
ReLU + map-cut) on 8 Trainium2 NeuronCores, data-parallel over the batch dim.

Fixed problem shapes: x (32,256,56,56) f32 -> out (32,512,54,54) f32.

Per-core device program (4 images each), fp8e4m3 matmul operands with
DoubleRow perf mode (two 128-deep contraction subtiles per instruction),
f32 PSUM:
  - depthwise 3x3 VALID conv: per 9-output-row chunk, 5 PSUM-accumulated
    DoubleRow matmuls, each packing TWO taps of the 3x3 stencil as the two
    k-subtiles (pairs chosen so the SBUF pair stride is even: odd strides
    fault the PE's DR ifmap fetch).  The rhs streams 504 contiguous
    elements (9 rows x 56 incl. 2 wrap cols); wrap cols land in psum cols
    the drains never read.  The 5th pair carries (1,1) plus a BIAS tap
    (all-ones rhs region times diag(b1)), so drains are a pure relu.
  - dw drains: relu(psum) -> fp8 Y (the pw DR rhs layout), split DVE
    (cin tile 0) / ACT (cin tile 1); per-map max via one DVE reduce per
    (image, cin tile) feeds the dw cut.  The cut is folded into
    per-image masked pw weights w2m = w2 * mask (and |w2| * mask).
  - pw cut masks are computed BEFORE the pw GEMM from the safe upper
    bound zUb[o] = sum_c |w2m[c,o]| * max_y[c] >= max_n |z'[o,n]| via one
    tiny DoubleRow matvec per (image, cout tile) into spare psum cols.
    mask2 = (zUb + b2 >= thresh) can only false-KEEP maps whose true
    values are < thresh (error <= 1e-3 abs), never false-cut, and keeps
    the all-zero canonical output exact.
  - pointwise 1x1: one DoubleRow matmul per 486-col chunk (K=256 = both
    cin tiles as the two k-subtiles); ACT drains each 2-chunk psum group
    once with relu + scale=mask2 + bias=mask2*b2 fused -> final bf16 z,
    stored per group (host upcasts to f32).  No second pass over z.
Pitfalls honored: no gpsimd bulk math (it is a slow DSP), no per-partition
AP scalars on bf16/fp8 DVE ops (PTR operand fetch is ~100x slow; masked
multiplies use stride-0 broadcast tensor operands instead), matmul out
<= one psum bank, DR pair strides even.
BatchNorm (inference) is folded into the conv weights/biases on the host.
"""

import os

os.environ.setdefault("NEURON_RT_RESET_CORES", "1")

import ml_dtypes
import numpy as np

import concourse.bacc as bacc
import concourse.bass as bass
import concourse.mybir as mybir
import concourse.tile as tile
from concourse.ap import AP
from concourse.bass_utils import run_bass_kernel_spmd

EPS = 1e-5
DW_THRESH = 4.0
PW_THRESH = 0.001

B, CIN, COUT, H, W = 32, 256, 512, 56, 56
HO, WO = 54, 54
NPIX = HO * WO          # 2916
NCORES = 8
BPC = B // NCORES       # 4 images per core
P = 128                 # partitions
KT = CIN // P           # 2 cin tiles
MT = COUT // P          # 4 cout tiles
NCH = 6                 # chunks per map: 6 x (9 rows x 54 cols)
CHROWS = HO // NCH      # 9
CHUNK = CHROWS * WO     # 486 output cols per chunk
FLAT = CHROWS * W       # 504 streamed cols per chunk (incl. 2 wrap cols)
XLEN = H * W            # 3136
ONES0 = XLEN + 1        # odd base of the ones region (bias tap rhs)
XPAD = XLEN + 8 + FLAT  # fp8 image + ones pad
BANK = 512

F32 = mybir.dt.float32
FP8 = mybir.dt.float8e4
BF16 = mybir.dt.bfloat16
DR = mybir.MatmulPerfMode.DoubleRow
ALU = mybir.AluOpType
AF = mybir.ActivationFunctionType

# tap pairs for the 5 DoubleRow depthwise matmuls (flat offset of tap
# (di,dj) = 56*di + dj; the pair stride o1-o0 must be EVEN).
TAP_PAIRS = [((0, 0), (0, 2)), ((1, 0), (1, 2)), ((2, 0), (2, 2)),
             ((0, 1), (2, 1)), ((1, 1), None)]

_cached_nc = None


def _build_program():
    nc = bacc.Bacc("TRN2", target_bir_lowering=False, debug=False)

    xs = nc.dram_tensor("xs", [BPC, CIN, XLEN], FP8, kind="ExternalInput").ap()
    dwp = nc.dram_tensor("dwp", [P, KT, 5, 2, P], FP8, kind="ExternalInput").ap()
    w2t = nc.dram_tensor("w2t", [P, KT, COUT], FP8, kind="ExternalInput").ap()
    uw2t = nc.dram_tensor("uw2t", [P, KT, COUT], FP8, kind="ExternalInput").ap()
    b2t = nc.dram_tensor("b2t", [P, MT], F32, kind="ExternalInput").ap()
    zs = nc.dram_tensor("zs", [BPC, COUT, NPIX], BF16, kind="ExternalOutput").ap()

    with tile.TileContext(nc) as tc:
        with (
            tc.tile_pool(name="consts", bufs=1) as consts,
            tc.tile_pool(name="xp", bufs=8) as xp,
            tc.tile_pool(name="yp", bufs=2) as yp,
            tc.tile_pool(name="zp", bufs=3) as zp,
            tc.tile_pool(name="wm", bufs=2) as wmp,
            tc.tile_pool(name="st", bufs=48) as st,
            tc.tile_pool(name="psdw", bufs=2, space="PSUM") as psdw,
            tc.tile_pool(name="pspw", bufs=3, space="PSUM") as pspw,
        ):
            xtiles = {}

            def load_x(b, k):
                X = xp.tile([P, XPAD], FP8, name="X")
                nc.sync.dma_start(out=X[:, 0:XLEN],
                                  in_=xs[b, k * P:(k + 1) * P, :])
                # ones region for the bias tap (covers any chunk's base)
                nc.gpsimd.memset(X[:, XLEN:XPAD], 1.0)
                xtiles[b, k] = X

            # first image's inputs + dw weights gate the first matmul
            load_x(0, 0)
            dwsb = consts.tile([P, KT, 5, 2, P], FP8)
            nc.sync.dma_start(out=dwsb, in_=dwp)
            load_x(0, 1)
            w2sb = consts.tile([P, KT, COUT], FP8)
            nc.sync.dma_start(out=w2sb, in_=w2t)
            uw2sb = consts.tile([P, KT, COUT], FP8)
            nc.sync.dma_start(out=uw2sb, in_=uw2t)
            b2sb = consts.tile([P, MT], F32)
            nc.sync.dma_start(out=b2sb, in_=b2t)
            for b in range(BPC):
                for k in range(KT):
                    if (b, k) not in xtiles:
                        load_x(b, k)

            def dw_rhs(X, pair, n):
                """[P, 2, 504] DoubleRow rhs: two tap-shifted flat views."""
                t0, t1 = TAP_PAIRS[pair]
                o0 = FLAT * n + W * t0[0] + t0[1]
                o1 = ONES0 if t1 is None else FLAT * n + W * t1[0] + t1[1]
                return AP(X.tensor, X.offset + o0,
                          [list(X.ap[0]), [o1 - o0, 2], [1, FLAT]])

            def emit_dw_chunk(b, k, n, Yk3):
                X = xtiles[b, k]
                P1 = psdw.tile([P, BANK], F32, name="P1")
                for p in range(5):
                    nc.tensor.matmul(
                        P1[:, 0:FLAT], lhsT=dwsb[:, k, p],
                        rhs=dw_rhs(X, p, n),
                        start=(p == 0), stop=(p == 4), perf_mode=DR)
                # drain valid cols (skip the 2 wrap cols per row):
                # y = relu(psum) -> fp8  (b1 was added by the bias tap)
                src = P1[:, 0:FLAT].rearrange(
                    "p (r w) -> p r w", w=W)[:, :, 0:WO]
                dst = Yk3[:, n].rearrange("p (r w) -> p r w", w=WO)
                if k == 0:
                    nc.vector.tensor_scalar(
                        out=dst, in0=src, scalar1=0.0, scalar2=None,
                        op0=ALU.max)
                else:
                    nc.scalar.activation(out=dst, in_=src, func=AF.Relu,
                                         bias=0.0, scale=1.0)

            Ys = {}
            masks = {}

            def dw_phase(b):
                Yi = yp.tile([P, KT, NPIX], FP8, name="Yi")
                Yi3 = Yi.rearrange("p k (c x) -> p k c x", x=CHUNK)
                w2m = wmp.tile([P, KT, COUT], FP8, name="w2m")
                uw2m = wmp.tile([P, KT, COUT], FP8, name="uw2m")
                Mv = st.tile([P, 4], FP8, name="Mv")
                # interleave the two cin tiles so their drains land on
                # different engines (k0 -> DVE, k1 -> ACT) and each psum
                # bank is recycled by whichever drain engine is free
                for n in range(NCH):
                    for k in range(KT):
                        emit_dw_chunk(b, k, n, Yi3[:, k])
                for k in range(KT):
                    # dw cut: zero the (image, channel) map if max(y) < 4
                    M = st.tile([P, 1], F32, name="M")
                    nc.vector.tensor_reduce(
                        M, Yi[:, k], axis=mybir.AxisListType.X, op=ALU.max)
                    mask1 = st.tile([P, 1], F32, name="mask1")
                    nc.vector.tensor_scalar(
                        out=mask1, in0=M, scalar1=DW_THRESH, scalar2=None,
                        op0=ALU.is_ge)
                    # masked pw weights (stride-0 broadcast operand: a
                    # per-partition AP scalar would hit the slow PTR path)
                    m1b = mask1.to_broadcast([P, COUT])
                    nc.vector.tensor_tensor(
                        out=w2m[:, k], in0=w2sb[:, k], in1=m1b, op=ALU.mult)
                    nc.vector.tensor_tensor(
                        out=uw2m[:, k], in0=uw2sb[:, k], in1=m1b, op=ALU.mult)
                    # per-channel y max, inflated 1.125x to stay an upper
                    # bound after fp8 rounding, at even slot stride for DR
                    nc.vector.tensor_scalar(
                        out=Mv[:, 2 * k:2 * k + 1], in0=M, scalar1=1.125,
                        scalar2=None, op0=ALU.mult)
                Ys[b] = (Yi, Yi3, w2m, uw2m, Mv)

            def pw_phase(b):
                Yi, Yi3, w2m, uw2m, Mv = Ys[b]
                # pw upper-bound matvecs: zUb[o] = sum_c |w2m[c,o]|*Mv[c],
                # all four into spare psum cols of ONE tile (m=0's first
                # group) so mask2/bias are ready before any pw drain
                G0 = pspw.tile([P, 2, BANK], F32, name="P2")
                rhs = AP(Mv.tensor, Mv.offset,
                         [list(Mv.ap[0]), [2, 2], [1, 1]])
                for m in range(MT):
                    c = CHUNK + 8 + m
                    nc.tensor.matmul(
                        G0[:, 0, c:c + 1],
                        lhsT=uw2m[:, :, m * P:(m + 1) * P], rhs=rhs,
                        start=True, stop=True, perf_mode=DR)
                mask2 = st.tile([P, MT], F32, name="mask2")
                b2m = st.tile([P, MT], F32, name="b2m")
                PU = st.tile([P, MT], F32, name="PU")
                nc.vector.tensor_scalar(
                    out=PU, in0=G0[:, 0, CHUNK + 8:CHUNK + 8 + MT],
                    scalar1=0.0, scalar2=None, op0=ALU.add)
                nc.vector.tensor_tensor(
                    out=mask2, in0=PU, in1=b2sb, op=ALU.add)
                nc.vector.tensor_scalar(
                    out=mask2, in0=mask2, scalar1=PW_THRESH, scalar2=None,
                    op0=ALU.is_ge)
                nc.vector.tensor_tensor(
                    out=b2m, in0=b2sb, in1=mask2, op=ALU.mult)

                for m in range(MT):
                    zf = zp.tile([P, NPIX], BF16, name="zf")
                    lhsT = w2m[:, :, m * P:(m + 1) * P]
                    for g in range(3):
                        P2 = G0 if (m == 0 and g == 0) else pspw.tile(
                            [P, 2, BANK], F32, name="P2")
                        for j in range(2):
                            n = 2 * g + j
                            nc.tensor.matmul(
                                P2[:, j, 0:CHUNK], lhsT=lhsT,
                                rhs=Yi3[:, :, n],
                                start=True, stop=True, perf_mode=DR)
                        # single fused drain: relu(mask2*psum + mask2*b2)
                        # = mask2 * relu(psum + b2) -> final bf16 z
                        dst = zf[:, g * 2 * CHUNK:(g + 1) * 2 * CHUNK] \
                            .rearrange("p (c x) -> p c x", x=CHUNK)
                        nc.scalar.activation(
                            out=dst, in_=P2[:, :, 0:CHUNK], func=AF.Relu,
                            bias=b2m[:, m:m + 1], scale=mask2[:, m:m + 1])
                        nc.sync.dma_start(
                            out=zs[b, m * P:(m + 1) * P,
                                   g * 2 * CHUNK:(g + 1) * 2 * CHUNK],
                            in_=zf[:, g * 2 * CHUNK:(g + 1) * 2 * CHUNK])

            # skewed emission: dw(b+1) is queued before pw(b) so the
            # in-order tensor queue never stalls on image b's cut masks
            dw_phase(0)
            for b in range(BPC):
                if b + 1 < BPC:
                    dw_phase(b + 1)
                pw_phase(b)
    nc.compile()
    return nc


def _prep_params(dw_w, dw_b, dw_gamma, dw_beta, dw_mean, dw_var,
                 pw_w, pw_b, pw_gamma, pw_beta, pw_mean, pw_var):
    dw_scale = dw_gamma / np.sqrt(dw_var + EPS)
    b1 = dw_b * dw_scale + dw_beta - dw_mean * dw_scale          # (256,)
    w1 = dw_w[:, 0] * dw_scale[:, None, None]                    # (256,3,3)

    dwp = np.zeros((P, KT, 5, 2, P), np.float32)
    idx = np.arange(P)
    for k in range(KT):
        for p, (t0, t1) in enumerate(TAP_PAIRS):
            dwp[idx, k, p, 0, idx] = w1[k * P:(k + 1) * P, t0[0], t0[1]]
            if t1 is None:
                dwp[idx, k, p, 1, idx] = b1[k * P:(k + 1) * P]
            else:
                dwp[idx, k, p, 1, idx] = w1[k * P:(k + 1) * P, t1[0], t1[1]]

    pw_scale = pw_gamma / np.sqrt(pw_var + EPS)
    b2 = pw_b * pw_scale + pw_beta - pw_mean * pw_scale          # (512,)
    w2 = pw_w * pw_scale[:, None]                                # (512,256)
    # w2t[ck, k, o] = w2[o, k*128+ck]
    w2t = np.ascontiguousarray(
        w2.T.reshape(KT, P, COUT).transpose(1, 0, 2)).astype(np.float32)
    b2t = np.ascontiguousarray(b2.reshape(MT, P).T).astype(np.float32)
    w2t8 = w2t.astype(ml_dtypes.float8_e4m3)
    uw2t8 = np.abs(w2t8.astype(np.float32)).astype(ml_dtypes.float8_e4m3)
    return (dwp.astype(ml_dtypes.float8_e4m3), w2t8, uw2t8, b2t)


def kernel(x, dw_w, dw_b, dw_gamma, dw_beta, dw_mean, dw_var,
           pw_w, pw_b, pw_gamma, pw_beta, pw_mean, pw_var):
    global _cached_nc
    x = np.ascontiguousarray(np.asarray(x, np.float32))
    args = [np.asarray(a, np.float32) for a in
            (dw_w, dw_b, dw_gamma, dw_beta, dw_mean, dw_var,
             pw_w, pw_b, pw_gamma, pw_beta, pw_mean, pw_var)]
    dwp, w2t, uw2t, b2t = _prep_params(*args)
    x8 = x.reshape(B, CIN, XLEN).astype(ml_dtypes.float8_e4m3)

    if _cached_nc is None:
        _cached_nc = _build_program()
    nc = _cached_nc

    in_maps = []
    for c in range(NCORES):
        in_maps.append({
            "xs": np.ascontiguousarray(x8[c * BPC:(c + 1) * BPC]),
            "dwp": dwp,
            "w2t": w2t,
            "uw2t": uw2t,
            "b2t": b2t,
        })
    res = run_bass_kernel_spmd(nc, in_maps, core_ids=list(range(NCORES)))
    out = np.concatenate(
        [res.results[c]["zs"].astype(np.float32) for c in range(NCORES)],
        axis=0)
    return out.reshape(B, COUT, HO, WO)


# revision 22
# speedup vs baseline: 1.1354x; 1.0025x over previous
"""Depthwise-separable conv2d block (dw3x3 + BN + ReLU + map-cut, pw1x1 + BN +
ReLU + map-cut) on 8 Trainium2 NeuronCores, data-parallel over the batch dim.

Fixed problem shapes: x (32,256,56,56) f32 -> out (32,512,54,54) f32.

Per-core device program (4 images each), fp8e4m3 matmul operands with
DoubleRow perf mode (two 128-deep contraction subtiles per instruction),
f32 PSUM:
  - depthwise 3x3 VALID conv: per 9-output-row chunk, 5 PSUM-accumulated
    DoubleRow matmuls, each packing TWO taps of the 3x3 stencil as the two
    k-subtiles (pairs chosen so the SBUF pair stride is even: odd strides
    fault the PE's DR ifmap fetch).  The rhs streams 504 contiguous
    elements (9 rows x 56 incl. 2 wrap cols); wrap cols land in psum cols
    the drains never read.  The 5th pair carries (1,1) plus a BIAS tap
    (all-ones rhs region times diag(b1)), so drains are a pure relu.
  - dw drains: relu(psum) -> fp8 Y (the pw DR rhs layout), split DVE
    (cin tile 0) / ACT (cin tile 1); per-map max via one DVE reduce per
    (image, cin tile) feeds the dw cut.  The cut is folded into
    per-image masked pw weights w2m = w2 * mask (and |w2| * mask).
  - pw cut masks are computed BEFORE the pw GEMM from the safe upper
    bound zUb[o] = sum_c |w2m[c,o]| * max_y[c] >= max_n |z'[o,n]| via one
    tiny DoubleRow matvec per (image, cout tile) into spare psum cols.
    mask2 = (zUb + b2 >= thresh) can only false-KEEP maps whose true
    values are < thresh (error <= 1e-3 abs), never false-cut, and keeps
    the all-zero canonical output exact.
  - pointwise 1x1: one DoubleRow matmul per 486-col chunk (K=256 = both
    cin tiles as the two k-subtiles); ACT drains each 2-chunk psum group
    once with relu + scale=mask2 + bias=mask2*b2 fused -> final bf16 z,
    stored per group (host upcasts to f32).  No second pass over z.
Pitfalls honored: no gpsimd bulk math (it is a slow DSP), no per-partition
AP scalars on bf16/fp8 DVE ops (PTR operand fetch is ~100x slow; masked
multiplies use stride-0 broadcast tensor operands instead), matmul out
<= one psum bank, DR pair strides even.
BatchNorm (inference) is folded into the conv weights/biases on the host.
"""

import os

os.environ.setdefault("NEURON_RT_RESET_CORES", "1")

import ml_dtypes
import numpy as np

import concourse.bacc as bacc
import concourse.bass as bass
import concourse.mybir as mybir
import concourse.tile as tile
from concourse.ap import AP
from concourse.bass_utils import run_bass_kernel_spmd

EPS = 1e-5
DW_THRESH = 4.0
PW_THRESH = 0.001

B, CIN, COUT, H, W = 32, 256, 512, 56, 56
HO, WO = 54, 54
NPIX = HO * WO          # 2916
NCORES = 8
BPC = B // NCORES       # 4 images per core
P = 128                 # partitions
KT = CIN // P           # 2 cin tiles
MT = COUT // P          # 4 cout tiles
NCH = 6                 # chunks per map: 6 x (9 rows x 54 cols)
CHROWS = HO // NCH      # 9
CHUNK = CHROWS * WO     # 486 output cols per chunk
FLAT = CHROWS * W       # 504 streamed cols per chunk (incl. 2 wrap cols)
XLEN = H * W            # 3136
ONES0 = XLEN + 1        # odd base of the ones region (bias tap rhs)
XPAD = XLEN + 8 + FLAT  # fp8 image + ones pad
BANK = 512

F32 = mybir.dt.float32
FP8 = mybir.dt.float8e4
BF16 = mybir.dt.bfloat16
DR = mybir.MatmulPerfMode.DoubleRow
ALU = mybir.AluOpType
AF = mybir.ActivationFunctionType

# tap pairs for the 5 DoubleRow depthwise matmuls (flat offset of tap
# (di,dj) = 56*di + dj; the pair stride o1-o0 must be EVEN).
TAP_PAIRS = [((0, 0), (0, 2)), ((1, 0), (1, 2)), ((2, 0), (2, 2)),
             ((0, 1), (2, 1)), ((1, 1), None)]

_cached_nc = None


def _build_program():
    nc = bacc.Bacc("TRN2", target_bir_lowering=False, debug=False)

    xs = nc.dram_tensor("xs", [BPC, CIN, XLEN], FP8, kind="ExternalInput").ap()
    dwp = nc.dram_tensor("dwp", [P, KT, 5, 2, P], FP8, kind="ExternalInput").ap()
    w2t = nc.dram_tensor("w2t", [P, KT, COUT], FP8, kind="ExternalInput").ap()
    uw2t = nc.dram_tensor("uw2t", [P, KT, COUT], FP8, kind="ExternalInput").ap()
    b2t = nc.dram_tensor("b2t", [P, MT], F32, kind="ExternalInput").ap()
    zs = nc.dram_tensor("zs", [BPC, COUT, NPIX], BF16, kind="ExternalOutput").ap()

    with tile.TileContext(nc) as tc:
        with (
            tc.tile_pool(name="consts", bufs=1) as consts,
            tc.tile_pool(name="xp", bufs=8) as xp,
            tc.tile_pool(name="yp", bufs=2) as yp,
            tc.tile_pool(name="zp", bufs=3) as zp,
            tc.tile_pool(name="wm", bufs=2) as wmp,
            tc.tile_pool(name="st", bufs=48) as st,
            tc.tile_pool(name="psdw", bufs=2, space="PSUM") as psdw,
            tc.tile_pool(name="pspw", bufs=3, space="PSUM") as pspw,
        ):
            xtiles = {}

            def load_x(b, k):
                X = xp.tile([P, XPAD], FP8, name="X")
                nc.sync.dma_start(out=X[:, 0:XLEN],
                                  in_=xs[b, k * P:(k + 1) * P, :])
                # ones region for the bias tap (covers any chunk's base)
                nc.gpsimd.memset(X[:, XLEN:XPAD], 1.0)
                xtiles[b, k] = X

            # first image's inputs + dw weights gate the first matmul
            load_x(0, 0)
            dwsb = consts.tile([P, KT, 5, 2, P], FP8)
            nc.sync.dma_start(out=dwsb, in_=dwp)
            load_x(0, 1)
            w2sb = consts.tile([P, KT, COUT], FP8)
            nc.sync.dma_start(out=w2sb, in_=w2t)
            uw2sb = consts.tile([P, KT, COUT], FP8)
            nc.sync.dma_start(out=uw2sb, in_=uw2t)
            b2sb = consts.tile([P, MT], F32)
            nc.sync.dma_start(out=b2sb, in_=b2t)
            for b in range(BPC):
                for k in range(KT):
                    if (b, k) not in xtiles:
                        load_x(b, k)

            def dw_rhs(X, pair, n):
                """[P, 2, 504] DoubleRow rhs: two tap-shifted flat views."""
                t0, t1 = TAP_PAIRS[pair]
                o0 = FLAT * n + W * t0[0] + t0[1]
                o1 = ONES0 if t1 is None else FLAT * n + W * t1[0] + t1[1]
                return AP(X.tensor, X.offset + o0,
                          [list(X.ap[0]), [o1 - o0, 2], [1, FLAT]])

            def emit_dw_chunk(b, k, n, Yk3):
                X = xtiles[b, k]
                P1 = psdw.tile([P, BANK], F32, name="P1")
                for p in range(5):
                    nc.tensor.matmul(
                        P1[:, 0:FLAT], lhsT=dwsb[:, k, p],
                        rhs=dw_rhs(X, p, n),
                        start=(p == 0), stop=(p == 4), perf_mode=DR)
                # drain valid cols (skip the 2 wrap cols per row):
                # y = relu(psum) -> fp8  (b1 was added by the bias tap)
                src = P1[:, 0:FLAT].rearrange(
                    "p (r w) -> p r w", w=W)[:, :, 0:WO]
                dst = Yk3[:, n].rearrange("p (r w) -> p r w", w=WO)
                if k == 0:
                    nc.vector.tensor_scalar(
                        out=dst, in0=src, scalar1=0.0, scalar2=None,
                        op0=ALU.max)
                else:
                    nc.scalar.activation(out=dst, in_=src, func=AF.Relu,
                                         bias=0.0, scale=1.0)

            Ys = {}
            masks = {}

            def dw_phase(b):
                Yi = yp.tile([P, KT, NPIX], FP8, name="Yi")
                Yi3 = Yi.rearrange("p k (c x) -> p k c x", x=CHUNK)
                w2m = wmp.tile([P, KT, COUT], FP8, name="w2m")
                uw2m = wmp.tile([P, KT, COUT], FP8, name="uw2m")
                Mv = st.tile([P, 4], FP8, name="Mv")
                for k in range(KT):
                    for n in range(NCH):
                        emit_dw_chunk(b, k, n, Yi3[:, k])
                    # dw cut: zero the (image, channel) map if max(y) < 4
                    M = st.tile([P, 1], F32, name="M")
                    nc.vector.tensor_reduce(
                        M, Yi[:, k], axis=mybir.AxisListType.X, op=ALU.max)
                    mask1 = st.tile([P, 1], F32, name="mask1")
                    nc.vector.tensor_scalar(
                        out=mask1, in0=M, scalar1=DW_THRESH, scalar2=None,
                        op0=ALU.is_ge)
                    # masked pw weights (stride-0 broadcast operand: a
                    # per-partition AP scalar would hit the slow PTR path)
                    m1b = mask1.to_broadcast([P, COUT])
                    nc.vector.tensor_tensor(
                        out=w2m[:, k], in0=w2sb[:, k], in1=m1b, op=ALU.mult)
                    nc.vector.tensor_tensor(
                        out=uw2m[:, k], in0=uw2sb[:, k], in1=m1b, op=ALU.mult)
                    # per-channel y max, inflated 1.125x to stay an upper
                    # bound after fp8 rounding, at even slot stride for DR
                    nc.vector.tensor_scalar(
                        out=Mv[:, 2 * k:2 * k + 1], in0=M, scalar1=1.125,
                        scalar2=None, op0=ALU.mult)
                Ys[b] = (Yi, Yi3, w2m, uw2m, Mv)

            def pw_phase(b):
                Yi, Yi3, w2m, uw2m, Mv = Ys[b]
                # pw upper-bound matvecs: zUb[o] = sum_c |w2m[c,o]|*Mv[c],
                # all four into spare psum cols of ONE tile (m=0's first
                # group) so mask2/bias are ready before any pw drain
                G0 = pspw.tile([P, 2, BANK], F32, name="P2")
                rhs = AP(Mv.tensor, Mv.offset,
                         [list(Mv.ap[0]), [2, 2], [1, 1]])
                for m in range(MT):
                    c = CHUNK + 8 + m
                    nc.tensor.matmul(
                        G0[:, 0, c:c + 1],
                        lhsT=uw2m[:, :, m * P:(m + 1) * P], rhs=rhs,
                        start=True, stop=True, perf_mode=DR)
                mask2 = st.tile([P, MT], F32, name="mask2")
                b2m = st.tile([P, MT], F32, name="b2m")
                PU = st.tile([P, MT], F32, name="PU")
                nc.vector.tensor_scalar(
                    out=PU, in0=G0[:, 0, CHUNK + 8:CHUNK + 8 + MT],
                    scalar1=0.0, scalar2=None, op0=ALU.add)
                nc.vector.tensor_tensor(
                    out=mask2, in0=PU, in1=b2sb, op=ALU.add)
                nc.vector.tensor_scalar(
                    out=mask2, in0=mask2, scalar1=PW_THRESH, scalar2=None,
                    op0=ALU.is_ge)
                nc.vector.tensor_tensor(
                    out=b2m, in0=b2sb, in1=mask2, op=ALU.mult)

                for m in range(MT):
                    zf = zp.tile([P, NPIX], BF16, name="zf")
                    lhsT = w2m[:, :, m * P:(m + 1) * P]
                    for g in range(3):
                        P2 = G0 if (m == 0 and g == 0) else pspw.tile(
                            [P, 2, BANK], F32, name="P2")
                        for j in range(2):
                            n = 2 * g + j
                            nc.tensor.matmul(
                                P2[:, j, 0:CHUNK], lhsT=lhsT,
                                rhs=Yi3[:, :, n],
                                start=True, stop=True, perf_mode=DR)
                        # single fused drain: relu(mask2*psum + mask2*b2)
                        # = mask2 * relu(psum + b2) -> final bf16 z
                        dst = zf[:, g * 2 * CHUNK:(g + 1) * 2 * CHUNK] \
                            .rearrange("p (c x) -> p c x", x=CHUNK)
                        nc.scalar.activation(
                            out=dst, in_=P2[:, :, 0:CHUNK], func=AF.Relu,
                            bias=b2m[:, m:m + 1], scale=mask2[:, m:m + 1])
                        nc.sync.dma_start(
                            out=zs[b, m * P:(m + 1) * P,
                                   g * 2 * CHUNK:(g + 1) * 2 * CHUNK],
                            in_=zf[:, g * 2 * CHUNK:(g + 1) * 2 * CHUNK])

            # skewed emission: dw(b+1) is queued before pw(b) so the
            # in-order tensor queue never stalls on image b's cut masks
            dw_phase(0)
            for b in range(BPC):
                if b + 1 < BPC:
                    dw_phase(b + 1)
                pw_phase(b)
    nc.compile()
    return nc


def _prep_params(dw_w, dw_b, dw_gamma, dw_beta, dw_mean, dw_var,
                 pw_w, pw_b, pw_gamma, pw_beta, pw_mean, pw_var):
    dw_scale = dw_gamma / np.sqrt(dw_var + EPS)
    b1 = dw_b * dw_scale + dw_beta - dw_mean * dw_scale          # (256,)
    w1 = dw_w[:, 0] * dw_scale[:, None, None]                    # (256,3,3)

    dwp = np.zeros((P, KT, 5, 2, P), np.float32)
    idx = np.arange(P)
    for k in range(KT):
        for p, (t0, t1) in enumerate(TAP_PAIRS):
            dwp[idx, k, p, 0, idx] = w1[k * P:(k + 1) * P, t0[0], t0[1]]
            if t1 is None:
                dwp[idx, k, p, 1, idx] = b1[k * P:(k + 1) * P]
            else:
                dwp[idx, k, p, 1, idx] = w1[k * P:(k + 1) * P, t1[0], t1[1]]

    pw_scale = pw_gamma / np.sqrt(pw_var + EPS)
    b2 = pw_b * pw_scale + pw_beta - pw_mean * pw_scale          # (512,)
    w2 = pw_w * pw_scale[:, None]                                # (512,256)
    # w2t[ck, k, o] = w2[o, k*128+ck]
    w2t = np.ascontiguousarray(
        w2.T.reshape(KT, P, COUT).transpose(1, 0, 2)).astype(np.float32)
    b2t = np.ascontiguousarray(b2.reshape(MT, P).T).astype(np.float32)
    w2t8 = w2t.astype(ml_dtypes.float8_e4m3)
    uw2t8 = np.abs(w2t8.astype(np.float32)).astype(ml_dtypes.float8_e4m3)
    return (dwp.astype(ml_dtypes.float8_e4m3), w2t8, uw2t8, b2t)


def kernel(x, dw_w, dw_b, dw_gamma, dw_beta, dw_mean, dw_var,
           pw_w, pw_b, pw_gamma, pw_beta, pw_mean, pw_var):
    global _cached_nc
    x = np.ascontiguousarray(np.asarray(x, np.float32))
    args = [np.asarray(a, np.float32) for a in
            (dw_w, dw_b, dw_gamma, dw_beta, dw_mean, dw_var,
             pw_w, pw_b, pw_gamma, pw_beta, pw_mean, pw_var)]
    dwp, w2t, uw2t, b2t = _prep_params(*args)
    x8 = x.reshape(B, CIN, XLEN).astype(ml_dtypes.float8_e4m3)

    if _cached_nc is None:
        _cached_nc = _build_program()
    nc = _cached_nc

    in_maps = []
    for c in range(NCORES):
        in_maps.append({
            "xs": np.ascontiguousarray(x8[c * BPC:(c + 1) * BPC]),
            "dwp": dwp,
            "w2t": w2t,
            "uw2t": uw2t,
            "b2t": b2t,
        })
    res = run_bass_kernel_spmd(nc, in_maps, core_ids=list(range(NCORES)))
    out = np.concatenate(
        [res.results[c]["zs"].astype(np.float32) for c in range(NCORES)],
        axis=0)
    return out.reshape(B, COUT, HO, WO)
